# revision 10
# baseline (speedup 1.0000x reference)
"""GAT-style attention layer (gnn_message_passing) on 8 TRN2 NeuronCores.

Math (reference):
    xf  = X @ W.T                          [N, F1]
    s   = xf @ a0   (att_self,  per-row i)
    t   = xf @ a1   (att_neigh, per-col j)
    att[i,j]   = LeakyReLU_0.2(s_i + t_j)
    E[i,j]     = A[i,j] * exp(att[i,j])
    S_j        = sum_i E[i,j]              (softmax axis=0 denominator)
    out[i,g]   = sum_j E[i,j] * xf[j,g] / S_j

Sharding: 1D column (j) shard across 8 cores; core r owns j in
[r*1024, (r+1)*1024). The axis=0 softmax denominator is core-local.

v2 design (vs. the transpose-on-device baseline):
  * The host passes AshT = ((A[:, loc].T) - 1) * BIG as fp16
    [JL, N] — already TRANSPOSED (j on rows) and affine-folded so
    edges are 0 and non-edges -BIG.  The device does NO PE transposes
    and no PSUM round-trip for the big tensor.
  * s_i must be added along the FREE axis, so the kernel builds
    s_bcast [128, N] (s broadcast to every partition) with PE matmuls:
    stationary = (W.T@a0) replicated across 128 columns (host input
    WsB), moving = replicated fp16 X.T — out[p, i] = s_i for every p.
    This replaces the baseline's AllGather of s entirely.
  * One custom DVE op (registered into concourse.dve_ops at import)
    computes   w = max(z, 0.2*z),  z = (at + t_j) + s_i
    in a SINGLE 1x pass (4 ALU slices): in0 = AshT tile (in-place),
    in1 = s_bcast, s0 = t column (per-partition), imm2 = 0.2.
    The baseline needed 3-4 DVE instructions for the same elements.
  * ACT does E = Exp(w) with accum_out giving the per-j row sums
    (softmax denominators) for free.
  * Aggregation runs PER-jt into a persistent PSUM accumulator
    [128, 64*64] (all 8 banks): after jt's exp, xfn = xf*1/S, then 64
    matmuls ag[b] += ET.T chunk @ xfn, start at jt==0, stop at jt==7.
    This hides the aggregation under the stream; the tail is only
    PSUM->SBUF copies, one 2MB DMA, one ReduceScatter.

Per-core engine budget (errata-adjusted cost model):
  DVE  8 x (58+8192)      = 66k cyc @0.96GHz = 69us   <- bottleneck
  ACT  8 x (352+8192)     = 68k cyc @1.2GHz  = 57us
  DMA  16MB A + 5MB X/misc                   = 59us
  PE   512 agg matmuls + prefix              = 25us
"""

import sys

sys.path.insert(0, "/opt/trn_rl_repo")

import numpy as np
import ml_dtypes

_f8np = ml_dtypes.float8_e5m2

import concourse.bass as bass
import concourse.mybir as mybir
from concourse import bacc, tile
from concourse.bass_utils import run_bass_kernel_spmd

N, F, F1 = 8192, 256, 64
NCORES = 8
JL = N // NCORES      # 1024 local columns per core
NT = N // 128         # 64 node tiles (i-tiles)
JT = JL // 128        # 8 local j-tiles per core
FE = F1 + 2           # xf extended with s,t columns
BIG = 32768.0         # additive mask magnitude (fp8e5-exact)

f32 = mybir.dt.float32
bf16 = mybir.dt.bfloat16
f16 = mybir.dt.float16
f8 = mybir.dt.float8e5
Alu = mybir.AluOpType
AF = mybir.ActivationFunctionType


# --------------------------------------------------------------------------
# Custom DVE op: w = max(z, imm2*z), z = (in0 + s0) + in1.  One 1x pass
# replaces the baseline's tensor_scalar + tensor_tensor + tensor_scalar +
# tensor_tensor chain.  Registered into concourse.dve_ops' name->row map at
# import (the documented extension point; row 5-bit field has free slots).
# --------------------------------------------------------------------------
_LRELU_OP = None


def _register_lrelu_op():
    global _LRELU_OP
    if _LRELU_OP is not None:
        return _LRELU_OP
    import concourse.dve_ops as DOPS
    from concourse.dve_spec import C0, C2, Spec, Src0, Src1, lower, maxx
    from concourse.dve_uop import DveOpSpec

    name = "LRELU_ADD2_ANT"
    if name in DOPS.CUSTOM_DVE_SPECS:
        _LRELU_OP = next(op for op in DOPS.OPS if op.name == name)
        return _LRELU_OP

    z = (Src0 + C0) + Src1

    def _ref(in0, in1, s0, s1, imm2):
        zf = in0.astype(np.float32) + s0 + in1.astype(np.float32)
        return np.maximum(zf, zf * imm2)

    spec = Spec(body=maxx(z, z * C2), reference=_ref)

    row = DOPS._CUSTOM_DVE_ROW_BASE + len(DOPS.OPS)
    assert row < 0x20
    DOPS._SUB_OPCODE_FOR_NAME[name] = row
    shas = {}
    for ver in ("v3", "v4"):
        uops = lower(spec, ver=ver)
        shas[ver] = DveOpSpec(
            name=name, opcode=row, uops=uops, rd1_en=True
        ).sha(ver)
    op = DOPS.DveOp(name, spec, subdim=False, uops_sha=shas)
    DOPS.OPS.append(op)
    DOPS.CUSTOM_DVE_SPECS[name] = spec
    _LRELU_OP = op
    return op


def build_graph(n=N, ncores=NCORES, use_collective=True, reps=1):
    N_, NCORES_ = n, ncores
    JL_ = N_ // NCORES_
    NT_ = N_ // 128
    JT_ = JL_ // 128
    SCH = 2048                  # s_bcast build chunk (columns)
    NSCH = N_ // SCH
    lrelu = _register_lrelu_op()
    nc = bacc.Bacc("TRN2", target_bir_lowering=False, num_devices=NCORES_)

    AshT_d = nc.dram_tensor("AshT", [JL_, N_], f8, kind="ExternalInput")
    XTl_d = nc.dram_tensor("XTloc", [F, JL_], f32, kind="ExternalInput")
    XTf_d = nc.dram_tensor("XTfull", [F, N_], f16, kind="ExternalInput")
    WTe_d = nc.dram_tensor("WTe", [F, FE], f32, kind="ExternalInput")
    WsB_d = nc.dram_tensor("WsB", [F, 128], f16, kind="ExternalInput")
    # fp16 output + fp16 RS partials: halves the ReduceScatter bytes (the
    # single biggest exposed cost) and the output DMA; host upcasts.
    out_d = nc.dram_tensor("out", [JL_, F1], f16, kind="ExternalOutput")

    with tile.TileContext(nc) as tc:
        with (
            tc.tile_pool(name="persist", bufs=1) as P,
            tc.tile_pool(name="dram", bufs=1, space="DRAM") as DR,
        ):
            WTe_sb = P.tile([128, 2 * FE], f32)
            nc.sync.dma_start(WTe_sb[:, 0:FE], WTe_d[0:128, :])
            nc.sync.dma_start(WTe_sb[:, FE : 2 * FE], WTe_d[128:256, :])
            wsb_sb = P.tile([128, 2 * 128], f16)
            nc.sync.dma_start(wsb_sb[:, 0:128], WsB_d[0:128, :])
            nc.sync.dma_start(wsb_sb[:, 128:256], WsB_d[128:256, :])

            s_bcast = P.tile([128, N_], f16)
            xf_loc = P.tile([128, JT_ * FE], f32)
            cs = P.tile([128, JT_], f32)
            rinv = P.tile([128, JT_], f32)
            xfn = P.tile([128, JT_ * F1], bf16)

            partials_d = [
                DR.tile([N_, F1], f16, name=f"part{h}") for h in (0, 1)
            ]
            rs_outs = [
                DR.tile([JL_, F1], f16, name=f"rso{h}") for h in (0, 1)
            ]

            for rep_ in range(reps):
                # ===== phase 0: local features xf + s broadcast row ======
                with (
                    tc.tile_pool(name="xstage", bufs=1) as XS,
                    tc.tile_pool(name="xfps", bufs=2, space="PSUM") as XFP,
                    tc.tile_pool(name="scps", bufs=4, space="PSUM") as SCP,
                ):
                    xtl = XS.tile([128, 2 * JL_], f32, name="xtl")
                    nc.sync.dma_start(xtl[:, 0:JL_], XTl_d[0:128, :])
                    nc.sync.dma_start(xtl[:, JL_ : 2 * JL_], XTl_d[128:256, :])
                    for jt in range(JT_):
                        xfp = XFP.tile([128, FE], f32, name="xfp", bufs=2)
                        nc.tensor.matmul(
                            xfp[:],
                            xtl[:, jt * 128 : (jt + 1) * 128],
                            WTe_sb[:, 0:FE],
                            start=True,
                            stop=False,
                        )
                        nc.tensor.matmul(
                            xfp[:],
                            xtl[:, JL_ + jt * 128 : JL_ + (jt + 1) * 128],
                            WTe_sb[:, FE : 2 * FE],
                            start=False,
                            stop=True,
                        )
                        nc.vector.tensor_copy(
                            xf_loc[:, jt * FE : (jt + 1) * FE], xfp[:]
                        )

                    # s_bcast[p, i] = s_i for every p: stationary = ws
                    # replicated over 128 cols, moving = full fp16 X.T.
                    # Chunked so the stream's first DVE op can start as
                    # soon as the first chunks land.
                    xtf = XS.tile([128, 2 * N_], f16, name="xtf")
                    for c in range(NSCH):
                        for h in (0, 1):
                            nc.sync.dma_start(
                                xtf[:, h * N_ + c * SCH : h * N_ + (c + 1) * SCH],
                                XTf_d[h * 128 : (h + 1) * 128, c * SCH : (c + 1) * SCH],
                            )
                        for q in range(SCH // 512):
                            col = c * SCH + q * 512
                            scp = SCP.tile([128, 512], f32, name="scp", bufs=4)
                            nc.tensor.matmul(
                                scp[:],
                                wsb_sb[:, 0:128],
                                xtf[:, col : col + 512],
                                start=True,
                                stop=False,
                            )
                            nc.tensor.matmul(
                                scp[:],
                                wsb_sb[:, 128:256],
                                xtf[:, N_ + col : N_ + col + 512],
                                start=False,
                                stop=True,
                            )
                            if q % 2 == 0:
                                nc.scalar.copy(
                                    s_bcast[:, col : col + 512], scp[:]
                                )
                            else:
                                nc.vector.tensor_copy(
                                    s_bcast[:, col : col + 512], scp[:]
                                )

                # ===== stream: one fused DVE op + one exp per j-tile =====
                # Two j-halves: each half accumulates its aggregation in
                # PSUM, then its ReduceScatter is issued so RS(half 0)
                # overlaps the second half's stream; only RS(half 1) is
                # exposed at the tail.
                with (
                    tc.tile_pool(name="atp", bufs=3) as ATP,
                    tc.tile_pool(name="wtp", bufs=2) as WTP,
                    tc.tile_pool(name="etp", bufs=2) as ETP,
                    tc.tile_pool(name="aggps", bufs=1, space="PSUM") as AGP,
                    tc.tile_pool(name="ocp", bufs=2) as OCP,
                ):
                    ag = AGP.tile([128, NT_ * F1], f32, name="ag")
                    HJ = JT_ // 2
                    for half in (0, 1):
                        for jt in range(half * HJ, (half + 1) * HJ):
                            at = ATP.tile([128, N_], f8, name="at")
                            nc.sync.dma_start(
                                at[:], AshT_d[jt * 128 : (jt + 1) * 128, :]
                            )
                            t_ap = xf_loc[:, jt * FE + F1 + 1 : jt * FE + F1 + 2]
                            # w = max(z, 0.2z), z = at + t_j + s_i, in place.
                            # jt==0 is chunked to overlap the s_bcast build.
                            nch = NSCH if jt == 0 else 1
                            cw = N_ // nch
                            w = WTP.tile([128, N_], f16, name="w")
                            for c in range(nch):
                                nc.vector._custom_dve(
                                    lrelu,
                                    out=w[:, c * cw : (c + 1) * cw],
                                    in0=at[:, c * cw : (c + 1) * cw],
                                    in1=s_bcast[:, c * cw : (c + 1) * cw],
                                    s0=t_ap,
                                    imm2=0.2,
                                )
                            et = ETP.tile([128, N_], bf16, name="et")
                            nc.scalar.activation(
                                et[:],
                                w[:],
                                AF.Exp,
                                accum_out=cs[:, jt : jt + 1],
                            )
                            nc.vector.reciprocal(
                                rinv[:, jt : jt + 1], cs[:, jt : jt + 1]
                            )
                            nc.vector.tensor_scalar(
                                xfn[:, jt * F1 : (jt + 1) * F1],
                                xf_loc[:, jt * FE : jt * FE + F1],
                                rinv[:, jt : jt + 1],
                                None,
                                Alu.mult,
                            )
                            for b in range(NT_):
                                # start=True clears has_written for the WHOLE
                                # bank: issue it only on the first region of
                                # each bank per half; later regions' first
                                # writes overwrite-then-accumulate.
                                nc.tensor.matmul(
                                    ag[:, b * F1 : (b + 1) * F1],
                                    et[:, b * 128 : (b + 1) * 128],
                                    xfn[:, jt * F1 : (jt + 1) * F1],
                                    start=(jt == half * HJ and b % 8 == 0),
                                    stop=(jt == (half + 1) * HJ - 1),
                                )
                        stage = OCP.tile([128, NT_ * F1], f16, name="stage")
                        for k in range(8):
                            sl = slice(k * 512, (k + 1) * 512)
                            if k % 2 == 0:
                                nc.scalar.copy(stage[:, sl], ag[:, sl])
                            else:
                                nc.vector.tensor_copy(stage[:, sl], ag[:, sl])
                        nc.sync.dma_start(
                            partials_d[half][:].rearrange(
                                "(b p) g -> p b g", p=128
                            ),
                            stage[:].rearrange("p (b g) -> p b g", g=F1),
                        )
                        if use_collective:
                            nc.gpsimd.collective_compute(
                                "ReduceScatter",
                                Alu.add,
                                replica_groups=[list(range(NCORES_))],
                                ins=[partials_d[half][:].opt()],
                                outs=[rs_outs[half][:].opt()],
                            )

                    # ===== tail: out = rsA + rsB ==========================
                    with tc.tile_pool(name="fin", bufs=1) as FIN:
                        if use_collective:
                            ra = FIN.tile([128, JL_ * F1 // 128], f16, name="ra")
                            rb = FIN.tile([128, JL_ * F1 // 128], f16, name="rb")
                            nc.sync.dma_start(
                                ra[:],
                                rs_outs[0][:].rearrange(
                                    "(p q) g -> p (q g)", p=128
                                ),
                            )
                            nc.sync.dma_start(
                                rb[:],
                                rs_outs[1][:].rearrange(
                                    "(p q) g -> p (q g)", p=128
                                ),
                            )
                            nc.vector.tensor_tensor(
                                ra[:], ra[:], rb[:], Alu.add
                            )
                            nc.sync.dma_start(
                                out_d[:].rearrange("(p q) g -> p (q g)", p=128),
                                ra[:],
                            )
                        else:
                            nc.sync.dma_start(
                                out_d[:], partials_d[1][0:JL_, :]
                            )

    nc.compile()
    return nc


_GRAPH = None


def make_in_maps(X, A, W, a):
    X = np.asarray(X, dtype=np.float32)
    A = np.asarray(A, dtype=np.float32)
    W = np.asarray(W, dtype=np.float32)
    a = np.asarray(a, dtype=np.float32)

    WT = W.T.astype(np.float32)                               # [256, 64]
    WTe = np.concatenate([WT, WT @ a[0], WT @ a[1]], axis=1)  # [256, 66]
    WTe = np.ascontiguousarray(WTe, dtype=np.float32)
    WsB = np.ascontiguousarray(
        np.repeat(WT @ a[0], 128, axis=1), dtype=np.float16
    )                                                          # [256, 128]
    XTf = np.ascontiguousarray(X.T).astype(np.float16)         # [256, 8192]

    in_maps = []
    for r in range(NCORES):
        sl = slice(r * JL, (r + 1) * JL)
        in_maps.append(
            {
                "AshT": np.ascontiguousarray(
                    (A[:, sl].T - 1.0) * BIG
                ).astype(_f8np),
                "XTloc": np.ascontiguousarray(X[sl].T),
                "XTfull": XTf,
                "WTe": WTe,
                "WsB": WsB,
            }
        )
    return in_maps


def kernel(X, A, W, a):
    global _GRAPH
    if _GRAPH is None:
        _GRAPH = build_graph()
    nc = _GRAPH

    in_maps = make_in_maps(X, A, W, a)
    res = run_bass_kernel_spmd(nc, in_maps, list(range(NCORES)))
    out = np.concatenate(
        [res.results[r]["out"] for r in range(NCORES)], axis=0
    )
    return out.astype(np.float32)
